# revision 7
# baseline (speedup 1.0000x reference)
"""MultiHeadGAT (2-layer GATConv + 3 output heads) on 8 trn2 NeuronCores.

Strategy (self-contained, hardcoded for N=20000, E=320000, IN=128, H=4, F=64):
  - Nodes are sorted by in-degree and dealt round-robin to 8 cores; each
    core's 2500 nodes form 20 groups of 128 (degree-homogeneous), padded to
    2560 slots.  All per-core programs are identical (SPMD); per-core data
    (edge slots, indices) differs only in values.
  - Per layer, every core computes the full node-feature table
    rows = [h(256) | a_src(4) | a_dst(4) | pad] (1280B stride) into its local
    DRAM with dense matmuls, then processes only its own dst nodes:
    a dst-major padded-CSR edge phase fetches h[src] rows with the GPSIMD
    dma_gather custom op (one call per group), computes edge-softmax
    (exp without max-subtraction -- logits are O(1) here, mathematically
    identical) and aggregates with DVE multiply + strided reduce.
  - Layer boundary: layer-1 output (transposed) is returned per-core,
    reassembled on host, and fed replicated into the layer-2 launch.
"""
import os
import sys
import types

sys.path.insert(0, "/opt/trn_rl_repo")

import numpy as np

import concourse.bacc as bacc
import concourse.bass as bass
import concourse.mybir as mybir
import concourse.tile as tile
from concourse.masks import make_identity

F32 = mybir.dt.float32
I16 = mybir.dt.int16
AX = mybir.AxisListType
OP = mybir.AluOpType
AF = mybir.ActivationFunctionType

N, E, NC, P, G = 20000, 320000, 8, 128, 20
H, F, IN_DIM = 4, 64, 128
HF = H * F                      # 256
NPC = G * P                     # 2560 padded nodes per core
NPID = NC * NPC                 # 20480
ROW = 320                       # table row stride in f32 (1280 B)
PADBIAS = -200.0
MB = 4                          # dense macro-block: 4 x 128 nodes per DMA

_cache = {}


# ----------------------------------------------------------------------------
# host-side graph preprocessing (index manipulation only)
# ----------------------------------------------------------------------------
def _preprocess(edge_index):
    src = np.asarray(edge_index[0]).astype(np.int64)
    dst = np.asarray(edge_index[1]).astype(np.int64)
    deg = np.bincount(dst, minlength=N)

    order = np.argsort(-deg, kind="stable")
    rank = np.empty(N, np.int64)
    rank[order] = np.arange(N)
    core_of = rank % NC
    slot_of = rank // NC                       # [0, 2500)
    pid_of = core_of * NPC + slot_of
    g_of = slot_of // P
    p_of = slot_of % P

    maxdeg = np.zeros((NC, G), np.int64)
    np.maximum.at(maxdeg, (core_of, g_of), deg)
    Dg = maxdeg.max(axis=0) + 1                # slots per group (incl. self)
    off = np.concatenate([[0], np.cumsum(Dg)]).astype(np.int64)
    S = int(off[-1])

    es = np.argsort(dst, kind="stable")
    sd = dst[es]
    first = np.searchsorted(sd, np.arange(N), side="left")
    slot = 1 + (np.arange(E) - first[sd])      # 1-based within dst segment
    e_core = core_of[sd]
    e_p = p_of[sd]
    e_col = off[g_of[sd]] + slot

    colgrp = np.repeat(np.arange(G), Dg)       # group of each column [S]
    A_idx = (np.arange(NC)[:, None, None] * NPC
             + colgrp[None, :, None] * P
             + np.arange(P)[None, None, :]).astype(np.int16)
    A_idx[e_core, e_col, e_p] = pid_of[src[es]].astype(np.int16)

    PB = np.full((NC, S, P), PADBIAS, np.float32)
    PB[:, off[:-1], :] = 0.0
    PB[e_core, e_col, e_p] = 0.0

    RD = np.ones((NC, G, P), np.float32)
    RD[core_of, g_of, p_of] = 1.0 / np.maximum(deg, 1)

    # wrapped int16 gather-index tensor [P, 8*S] per core
    idx_w = np.empty((NC, P, 8 * S), np.int16)
    for c in range(NC):
        blocks = []
        for g in range(G):
            lst = A_idx[c, off[g]:off[g + 1], :].reshape(-1)   # i = s*128 + p
            w = lst.reshape(-1, 16).T                          # [16, 8*Dg]
            blocks.append(np.tile(w, (8, 1)))
        idx_w[c] = np.hstack(blocks)

    return dict(Dg=tuple(int(d) for d in Dg), off=off, S=S,
                pid_of=pid_of, core_of=core_of, slot_of=slot_of,
                idx_w=idx_w, PB=PB, RD=RD,
                e_core=e_core, e_col=e_col, e_p=e_p, es=es)


# ----------------------------------------------------------------------------
# shared edge phase (one group): returns t1 = (softmax-weighted sum) tile
# ----------------------------------------------------------------------------
def _edge_phase(nc, pools, tbl, idx_sb, ae_sb, g, off_g, Dg):
    sb, aggp = pools
    gt = sb.tile([P, Dg, ROW], F32, tag="gt")
    nc.gpsimd.dma_gather(
        out_ap=gt[:], in_ap=tbl[:, :],
        idxs_ap=idx_sb[:, 8 * off_g: 8 * (off_g + Dg)],
        num_idxs=P * Dg, num_idxs_reg=P * Dg, elem_size=ROW,
        single_packet=False)

    # logits y = a_s(gathered) + a_e' (incl self-loop mean + padbias)
    #            + a_d(own, from the self slot of the gathered rows)
    y = sb.tile([P, Dg, H], F32, tag="y")
    nc.vector.tensor_tensor(out=y[:], in0=gt[:, :, HF:HF + H],
                            in1=ae_sb[:, off_g:off_g + Dg, :], op=OP.add)
    nc.vector.tensor_tensor(
        out=y[:], in0=y[:],
        in1=gt[:, 0:1, HF + H:HF + 2 * H].to_broadcast([P, Dg, H]),
        op=OP.add)
    # leaky_relu(y) = 0.6*y + 0.4*|y|
    ab = sb.tile([P, Dg, H], F32, tag="ab")
    nc.scalar.activation(ab[:], y[:], AF.Abs, bias=0.0, scale=0.4)
    lr = sb.tile([P, Dg, H], F32, tag="lr")
    nc.vector.scalar_tensor_tensor(out=lr[:], in0=y[:], scalar=0.6, in1=ab[:],
                                   op0=OP.mult, op1=OP.add)
    ex = sb.tile([P, Dg, H], F32, tag="ex")
    nc.scalar.activation(ex[:], lr[:], AF.Exp)
    den = sb.tile([P, H], F32, tag="den")
    nc.vector.tensor_reduce(out=den[:],
                            in_=ex[:].rearrange("p s h -> p h s"),
                            axis=AX.X, op=OP.add)
    rden = sb.tile([P, H], F32, tag="rden")
    nc.vector.reciprocal(rden[:], den[:])

    # messages (in-place: scale gathered features by ex) and aggregation
    nc.vector.tensor_tensor(
        out=gt[:, :, 0:HF].rearrange("p s (h f) -> p s h f", h=H),
        in0=gt[:, :, 0:HF].rearrange("p s (h f) -> p s h f", h=H),
        in1=ex[:, :, :, None].to_broadcast([P, Dg, H, F]),
        op=OP.mult)
    agg = aggp.tile([P, HF], F32, tag="agg")
    nc.vector.tensor_reduce(
        out=agg[:],
        in_=gt[:, :, 0:HF].rearrange("p s f -> p f s"),
        axis=AX.X, op=OP.add)
    t1 = aggp.tile([P, HF], F32, tag="t1")
    nc.vector.tensor_tensor(out=t1[:].rearrange("p (h f) -> p h f", h=H),
                            in0=agg[:].rearrange("p (h f) -> p h f", h=H),
                            in1=rden[:, :, None].to_broadcast([P, H, F]),
                            op=OP.mult)
    return t1


def _ae_finalize(nc, sb, ae_sb, pb_sb, rdeg_sb, Dg, off, S):
    """Per group: a_e[slot0] = mean of real slots' a_e (pads are 0, so the
    plain sum * 1/deg is exact); then fold padbias into a_e."""
    for g in range(G):
        off_g, D = int(off[g]), int(Dg[g])
        tmp4 = sb.tile([P, H], F32, tag="aefix")
        nc.vector.tensor_reduce(
            out=tmp4[:],
            in_=ae_sb[:, off_g + 1: off_g + D, :].rearrange("p s h -> p h s"),
            axis=AX.X, op=OP.add)
        nc.vector.tensor_scalar_mul(
            out=ae_sb[:, off_g, :], in0=tmp4[:], scalar1=rdeg_sb[:, g:g + 1])
    nc.vector.tensor_tensor(
        out=ae_sb[:], in0=ae_sb[:],
        in1=pb_sb[:, :, None].to_broadcast([P, S, H]),
        op=OP.add)


def _ae_from_ea(nc, sb, ea_sb, wae, S):
    """a_e[p,s,h] = ea0*wae[0,h] + ea1*wae[1,h]; padbias kept separate."""
    ae_sb = sb.tile([P, S, H], F32, tag="ae")
    t = sb.tile([P, S], F32, tag="aetmp")
    for h in range(H):
        nc.vector.tensor_scalar_mul(out=t[:], in0=ea_sb[:, :, 1],
                                    scalar1=float(wae[1, h]))
        nc.vector.scalar_tensor_tensor(
            out=ae_sb[:, :, h], in0=ea_sb[:, :, 0], scalar=float(wae[0, h]),
            in1=t[:], op0=OP.mult, op1=OP.add)
    return ae_sb


def _bias_bcast(nc, sb, psp, ones_row, src_row, width):
    ps = psp.tile([P, width], F32, tag="bb_ps")
    nc.tensor.matmul(out=ps[:], lhsT=ones_row[:], rhs=src_row[:],
                     start=True, stop=True)
    out = sb.tile([P, width], F32, tag="bb_" + str(width))
    nc.vector.tensor_copy(out=out[:], in_=ps[:])
    return out


def _elu_parts(nc, sb, t2, width, tag):
    """returns m1 with elu(t2) = m1 - 1 (the -1 is folded by the caller)."""
    mn = sb.tile([P, width], F32, tag=tag + "mn")
    nc.vector.tensor_scalar_min(out=mn[:], in0=t2[:], scalar1=0.0)
    ef = sb.tile([P, width], F32, tag=tag + "ef")
    nc.scalar.activation(ef[:], mn[:], AF.Exp)
    rl = sb.tile([P, width], F32, tag=tag + "rl")
    nc.scalar.activation(rl[:], t2[:], AF.Relu)
    m1 = sb.tile([P, width], F32, tag=tag + "m1")
    nc.vector.tensor_tensor(out=m1[:], in0=ef[:], in1=rl[:], op=OP.add)
    return m1


# ----------------------------------------------------------------------------
# layer builders
# ----------------------------------------------------------------------------
def _build_layer1(Dg, off, S, wae1):
    nc = bacc.Bacc("TRN2", target_bir_lowering=False, debug=False,
                   num_devices=NC)
    xT = nc.dram_tensor("xT", [IN_DIM, NPID], F32, kind="ExternalInput")
    w1 = nc.dram_tensor("w1ext", [IN_DIM, HF + 2 * H], F32,
                        kind="ExternalInput")
    b1r = nc.dram_tensor("b1row", [1, HF], F32, kind="ExternalInput")
    idx = nc.dram_tensor("idx", [P, 8 * S], I16, kind="ExternalInput")
    ea = nc.dram_tensor("ea", [P, S, 2], F32, kind="ExternalInput")
    pb = nc.dram_tensor("pb", [P, S], F32, kind="ExternalInput")
    rdg = nc.dram_tensor("rdeg", [P, G], F32, kind="ExternalInput")
    h1T = nc.dram_tensor("h1T", [HF, NPC], F32, kind="ExternalOutput")
    tbl = nc.dram_tensor("tbl1", [NPID, ROW], F32, kind="Internal")

    KW = HF + 2 * H   # 264
    with tile.TileContext(nc) as tc:
        with tc.tile_pool(name="persist", bufs=1) as pp, \
             tc.tile_pool(name="sb", bufs=3) as sb, \
             tc.tile_pool(name="aggp", bufs=2) as aggp, \
             tc.tile_pool(name="dn", bufs=3) as dn, \
             tc.tile_pool(name="ps", bufs=2, space="PSUM") as psp, \
             tc.tile_pool(name="psT", bufs=2, space="PSUM") as psT:
            w1_sb = pp.tile([IN_DIM, KW], F32)
            nc.sync.dma_start(out=w1_sb[:], in_=w1[:, :])
            ones_row = pp.tile([1, P], F32)
            nc.vector.memset(ones_row[:], 1.0)
            b1_row = pp.tile([1, HF], F32)
            nc.sync.dma_start(out=b1_row[:], in_=b1r[:, :])
            b1_bc = _bias_bcast(nc, pp, psp, ones_row, b1_row, HF)
            ident = pp.tile([P, P], F32)
            make_identity(nc, ident[:])

            # dense: table rows [h | a_s | a_d] for all NPID nodes
            for nb in range(NPID // (MB * P)):          # 40 macro-blocks
                xt_big = dn.tile([P, MB * P], F32, tag="xt")
                nc.sync.dma_start(
                    out=xt_big[:], in_=xT[:, nb * MB * P:(nb + 1) * MB * P])
                row_big = dn.tile([P, MB, KW], F32, tag="row")
                for j in range(MB):
                    psd = psp.tile([P, KW], F32, tag="psd")
                    nc.tensor.matmul(out=psd[:],
                                     lhsT=xt_big[:, j * P:(j + 1) * P],
                                     rhs=w1_sb[:], start=True, stop=True)
                    nc.vector.tensor_copy(out=row_big[:, j, :], in_=psd[:])
                nc.sync.dma_start(
                    out=tbl[nb * MB * P:(nb + 1) * MB * P, 0:KW].rearrange(
                        "(j p) f -> p j f", j=MB),
                    in_=row_big[:])

            idx_sb = pp.tile([P, 8 * S], I16)
            nc.sync.dma_start(out=idx_sb[:], in_=idx[:, :])
            ea_sb = pp.tile([P, S, 2], F32)
            nc.sync.dma_start(out=ea_sb[:], in_=ea[:, :, :])
            pb_sb = pp.tile([P, S], F32)
            nc.sync.dma_start(out=pb_sb[:], in_=pb[:, :])
            rdeg_sb = pp.tile([P, G], F32)
            nc.sync.dma_start(out=rdeg_sb[:], in_=rdg[:, :])
            ae_sb = _ae_from_ea(nc, pp, ea_sb, wae1, S)
            _ae_finalize(nc, sb, ae_sb, pb_sb, rdeg_sb, Dg, off, S)

            h1T_acc = pp.tile([P, 2, G, P], F32)
            for g in range(G):
                t1 = _edge_phase(nc, (sb, aggp), tbl, idx_sb, ae_sb,
                                 g, int(off[g]), int(Dg[g]))
                t2 = aggp.tile([P, HF], F32, tag="t2")
                nc.vector.tensor_tensor(out=t2[:], in0=t1[:], in1=b1_bc[:],
                                        op=OP.add)
                m1 = _elu_parts(nc, sb, t2, HF, "e1")
                for kc in range(2):
                    tp = psT.tile([P, P], F32, tag="tp")
                    nc.tensor.transpose(out=tp[:],
                                        in_=m1[:, kc * P:(kc + 1) * P],
                                        identity=ident[:])
                    nc.vector.tensor_scalar_add(out=h1T_acc[:, kc, g, :],
                                                in0=tp[:], scalar1=-1.0)
            nc.sync.dma_start(
                out=h1T[:, :].rearrange("(kc p) (g s) -> p kc g s",
                                        kc=2, g=G),
                in_=h1T_acc[:])
    nc.compile()
    return nc


def _build_layer2(Dg, off, S, wae2):
    nc = bacc.Bacc("TRN2", target_bir_lowering=False, debug=False,
                   num_devices=NC)
    hT = nc.dram_tensor("hT", [HF, NPID], F32, kind="ExternalInput")
    w2 = nc.dram_tensor("w2ext", [HF, HF + 2 * H], F32, kind="ExternalInput")
    b2r = nc.dram_tensor("b2row", [1, F], F32, kind="ExternalInput")
    wcat = nc.dram_tensor("wcat", [F, 80], F32, kind="ExternalInput")
    bcat = nc.dram_tensor("bcat", [1, 80], F32, kind="ExternalInput")
    idx = nc.dram_tensor("idx", [P, 8 * S], I16, kind="ExternalInput")
    ea = nc.dram_tensor("ea", [P, S, 2], F32, kind="ExternalInput")
    pb = nc.dram_tensor("pb", [P, S], F32, kind="ExternalInput")
    rdg = nc.dram_tensor("rdeg", [P, G], F32, kind="ExternalInput")
    logits = nc.dram_tensor("logits", [NPC, 80], F32, kind="ExternalOutput")
    tbl = nc.dram_tensor("tbl2", [NPID, ROW], F32, kind="Internal")

    KW = HF + 2 * H
    with tile.TileContext(nc) as tc:
        with tc.tile_pool(name="persist", bufs=1) as pp, \
             tc.tile_pool(name="sb", bufs=3) as sb, \
             tc.tile_pool(name="aggp", bufs=2) as aggp, \
             tc.tile_pool(name="dn", bufs=3) as dn, \
             tc.tile_pool(name="ps", bufs=2, space="PSUM") as psp, \
             tc.tile_pool(name="psT", bufs=2, space="PSUM") as psT, \
             tc.tile_pool(name="psH", bufs=2, space="PSUM") as psH:
            w2_sb = [pp.tile([P, KW], F32, tag=f"w2_{k}", name=f"w2sb{k}")
                     for k in range(2)]
            for k in range(2):
                nc.sync.dma_start(out=w2_sb[k][:],
                                  in_=w2[k * P:(k + 1) * P, :])
            wcat_sb = pp.tile([F, 80], F32)
            nc.sync.dma_start(out=wcat_sb[:], in_=wcat[:, :])
            ones_row = pp.tile([1, P], F32)
            nc.vector.memset(ones_row[:], 1.0)
            b2_row = pp.tile([1, F], F32)
            nc.sync.dma_start(out=b2_row[:], in_=b2r[:, :])
            b2_bc = _bias_bcast(nc, pp, psp, ones_row, b2_row, F)
            bc_row = pp.tile([1, 80], F32)
            nc.sync.dma_start(out=bc_row[:], in_=bcat[:, :])
            bcat_bc = _bias_bcast(nc, pp, psp, ones_row, bc_row, 80)
            ident = pp.tile([P, P], F32)
            make_identity(nc, ident[:])

            for nb in range(NPID // (MB * P)):
                hT_big = [dn.tile([P, MB * P], F32, tag=f"ht{k}", name=f"htb{k}")
                          for k in range(2)]
                for k in range(2):
                    nc.sync.dma_start(
                        out=hT_big[k][:],
                        in_=hT[k * P:(k + 1) * P,
                               nb * MB * P:(nb + 1) * MB * P])
                row_big = dn.tile([P, MB, KW], F32, tag="row")
                for j in range(MB):
                    psd = psp.tile([P, KW], F32, tag="psd")
                    nc.tensor.matmul(out=psd[:],
                                     lhsT=hT_big[0][:, j * P:(j + 1) * P],
                                     rhs=w2_sb[0][:], start=True, stop=False)
                    nc.tensor.matmul(out=psd[:],
                                     lhsT=hT_big[1][:, j * P:(j + 1) * P],
                                     rhs=w2_sb[1][:], start=False, stop=True)
                    nc.vector.tensor_copy(out=row_big[:, j, :], in_=psd[:])
                nc.sync.dma_start(
                    out=tbl[nb * MB * P:(nb + 1) * MB * P, 0:KW].rearrange(
                        "(j p) f -> p j f", j=MB),
                    in_=row_big[:])

            idx_sb = pp.tile([P, 8 * S], I16)
            nc.sync.dma_start(out=idx_sb[:], in_=idx[:, :])
            ea_sb = pp.tile([P, S, 2], F32)
            nc.sync.dma_start(out=ea_sb[:], in_=ea[:, :, :])
            pb_sb = pp.tile([P, S], F32)
            nc.sync.dma_start(out=pb_sb[:], in_=pb[:, :])
            rdeg_sb = pp.tile([P, G], F32)
            nc.sync.dma_start(out=rdeg_sb[:], in_=rdg[:, :])
            ae_sb = _ae_from_ea(nc, pp, ea_sb, wae2, S)
            _ae_finalize(nc, sb, ae_sb, pb_sb, rdeg_sb, Dg, off, S)

            for g in range(G):
                t1 = _edge_phase(nc, (sb, aggp), tbl, idx_sb, ae_sb,
                                 g, int(off[g]), int(Dg[g]))
                hsum = aggp.tile([P, F], F32, tag="hsum")
                nc.vector.tensor_reduce(
                    out=hsum[:],
                    in_=t1[:].rearrange("p (h f) -> p f h", h=H),
                    axis=AX.X, op=OP.add)
                t3 = aggp.tile([P, F], F32, tag="t3")
                nc.vector.scalar_tensor_tensor(out=t3[:], in0=hsum[:],
                                               scalar=0.25, in1=b2_bc[:],
                                               op0=OP.mult, op1=OP.add)
                m1 = _elu_parts(nc, sb, t3, F, "e2")
                tp = psT.tile([P, P], F32, tag="tp")
                nc.tensor.transpose(out=tp[:F, :], in_=m1[:, :],
                                    identity=ident[:])
                gT = sb.tile([F, P], F32, tag="gT")
                nc.vector.tensor_scalar_add(out=gT[:], in0=tp[:F, :],
                                            scalar1=-1.0)
                pl = psH.tile([P, 80], F32, tag="pl")
                nc.tensor.matmul(out=pl[:], lhsT=gT[:], rhs=wcat_sb[:],
                                 start=True, stop=True)
                lo = sb.tile([P, 80], F32, tag="lo")
                nc.vector.tensor_tensor(out=lo[:], in0=pl[:], in1=bcat_bc[:],
                                        op=OP.add)
                nc.sync.dma_start(out=logits[g * P:(g + 1) * P, :],
                                  in_=lo[:])
    nc.compile()
    return nc


# ----------------------------------------------------------------------------
# entry point
# ----------------------------------------------------------------------------
def _install_ntff_hook():
    try:
        import antenv
        if "antenv.axon_hooks" not in sys.modules:
            mod = types.ModuleType("antenv.axon_hooks")
            _h = [None]
            mod.set_axon_ntff_profile_hook = lambda h: _h.__setitem__(0, h)
            mod.get_axon_ntff_profile_hook = lambda: _h[0]
            sys.modules["antenv.axon_hooks"] = mod
            antenv.axon_hooks = mod
            from trn_agent_boot.trn_boot import _ntff_profile_via_ctypes
            mod.set_axon_ntff_profile_hook(
                _ntff_profile_via_ctypes("/opt/axon/libaxon_pjrt.so"))
    except Exception:
        pass


def kernel(x, edge_index, edge_attr, W1, We1, as1, ad1, ae1, b1,
           W2, We2, as2, ad2, ae2, b2, Ww, bw, Wt, bt, Wa, ba):
    from concourse import bass_utils

    trace = os.environ.get("GAT_TRACE", "0") == "1"
    if trace:
        _install_ntff_hook()

    x = np.asarray(x, np.float32)
    edge_attr = np.asarray(edge_attr, np.float32)
    to32 = lambda a: np.asarray(a, np.float32)
    W1, We1, as1, ad1, ae1, b1 = map(to32, (W1, We1, as1, ad1, ae1, b1))
    W2, We2, as2, ad2, ae2, b2 = map(to32, (W2, We2, as2, ad2, ae2, b2))
    Ww, bw, Wt, bt, Wa, ba = map(to32, (Ww, bw, Wt, bt, Wa, ba))

    meta = _preprocess(edge_index)
    Dg, off, S = meta["Dg"], meta["off"], meta["S"]

    # edge_attr in slot layout, per core: [P, S, 2]
    EA = np.zeros((NC, P, S, 2), np.float32)
    EA[meta["e_core"], meta["e_p"], meta["e_col"]] = edge_attr[meta["es"]]

    # fused weights
    wae1 = np.einsum("ihf,hf->ih", We1.reshape(2, H, F), ae1)
    wae2 = np.einsum("ihf,hf->ih", We2.reshape(2, H, F), ae2)
    w1ext = np.concatenate(
        [W1, np.einsum("ihf,hf->ih", W1.reshape(IN_DIM, H, F), as1),
         np.einsum("ihf,hf->ih", W1.reshape(IN_DIM, H, F), ad1)], axis=1)
    w2ext = np.concatenate(
        [W2, np.einsum("ihf,hf->ih", W2.reshape(HF, H, F), as2),
         np.einsum("ihf,hf->ih", W2.reshape(HF, H, F), ad2)], axis=1)
    wcat = np.concatenate([Ww, Wt, Wa], axis=1)
    bcat = np.concatenate([bw, bt, ba])[None, :]

    key = (Dg, S)
    if key not in _cache:
        _cache.clear()
        _cache[key] = (_build_layer1(Dg, off, S, wae1),
                       _build_layer2(Dg, off, S, wae2))
    nc1, nc2 = _cache[key]

    X = np.zeros((NPID, IN_DIM), np.float32)
    X[meta["pid_of"]] = x
    xT = np.ascontiguousarray(X.T)

    in1 = []
    for c in range(NC):
        in1.append(dict(xT=xT, w1ext=w1ext, b1row=b1[None, :],
                        idx=meta["idx_w"][c],
                        ea=EA[c],
                        pb=np.ascontiguousarray(meta["PB"][c].T),
                        rdeg=np.ascontiguousarray(meta["RD"][c].transpose(1, 0))))
    r1 = bass_utils.run_bass_kernel_spmd(nc1, in1, core_ids=list(range(NC)),
                                         trace=trace)
    if trace:
        print(f"layer1 exec_time_ns: {r1.exec_time_ns}")
    hT_full = np.concatenate([r1.results[c]["h1T"] for c in range(NC)],
                             axis=1)

    in2 = []
    for c in range(NC):
        in2.append(dict(hT=hT_full, w2ext=w2ext, b2row=b2[None, :],
                        wcat=wcat, bcat=bcat,
                        idx=meta["idx_w"][c],
                        ea=EA[c],
                        pb=np.ascontiguousarray(meta["PB"][c].T),
                        rdeg=np.ascontiguousarray(meta["RD"][c].transpose(1, 0))))
    r2 = bass_utils.run_bass_kernel_spmd(nc2, in2, core_ids=list(range(NC)),
                                         trace=trace)
    if trace:
        print(f"layer2 exec_time_ns: {r2.exec_time_ns}")
    kernel.exec_time_ns = ((r1.exec_time_ns or 0) + (r2.exec_time_ns or 0)) \
        if trace else None

    full = np.empty((N, 80), np.float32)
    core_of, slot_of = meta["core_of"], meta["slot_of"]
    allc = np.stack([r2.results[c]["logits"] for c in range(NC)])
    full = allc[core_of, slot_of]
    return full[:, :50], full[:, 50:70], full[:, 70:80]
